# revision 1
# baseline (speedup 1.0000x reference)
"""FP8 dynamic-quantized linear (nn_FP8Linear) on 8 Trainium2 NeuronCores.

out = fp16((x_fp8 @ w_fp8.T) / (sx*sw)) + bias, with per-tensor dynamic
fp8-e4m3 quantization of x and weight (scale = FP8_MAX / amax).

Sharding: weight/bias split along out_features across 8 cores, x replicated.
Each core also receives a disjoint row-slice of x; per-core partial amaxes
are combined with one tiny AllReduce(max) so every core quantizes with the
global per-tensor scales (matching the reference exactly).

TRN fp8e4 (float8_e4m3) has max +-240 vs OCP e4m3fn's +-448, so the device
uses scale 224/amax == ref_scale/2: fp8 grids are self-similar under powers
of two, so device fp8 values are exactly half the reference's, and the
dequant multipliers (= 2x the reference's each) absorb the factor of 4.
"""

import time

import numpy as np

import concourse.bacc as bacc
import concourse.bass as bass
import concourse.bass_isa as bass_isa
import concourse.mybir as mybir
import concourse.tile as tile
from concourse.bass_utils import run_bass_kernel_spmd

F16 = mybir.dt.float16
F32 = mybir.dt.float32
F8 = mybir.dt.float8e4

NCORES = 8
EPS = 1e-12
# device-side quantization scale numerator: ref uses 448 (e4m3fn max); we use
# 224 so quantized values stay within TRN e4m3's +-240 normal range.
DEV_FP8_MAX = 224.0


def build_kernel(M=4096, K=4096, NSH=512, SW=1024, double_row=False):
    """Build + compile the per-core bass program.

    M tokens, K in_features, NSH out_features per core, SW m-stripe width.
    double_row: use fp8 DoubleRow matmuls (~1.8x PE throughput but the PE's
    doubled-row accumulation path adds ~1e-4 relative noise); False uses
    normal-mode fp8 matmuls whose f32 accumulation is bit-faithful.
    """
    KCH = K // 128      # k-chunks of 128
    KB = K // 256       # k-blocks of 256 (DoubleRow contracts 256/pass)
    NSTRIPES = M // SW
    MCH = SW // 128     # m-chunks per stripe
    MS = M // NCORES    # rows of the per-core amax slice of x
    assert MS * K % 128 == 0

    nc = bacc.Bacc("TRN2", target_bir_lowering=False, debug=False,
                   num_devices=NCORES)
    x = nc.dram_tensor("x", [M, K], F16, kind="ExternalInput").ap()
    xs = nc.dram_tensor("xs", [MS, K], F16, kind="ExternalInput").ap()
    w = nc.dram_tensor("w", [NSH, K], F16, kind="ExternalInput").ap()
    bias = nc.dram_tensor("bias", [1, NSH], F16, kind="ExternalInput").ap()
    out = nc.dram_tensor("out", [M, NSH], F16, kind="ExternalOutput").ap()

    with tile.TileContext(nc) as tc:
        with (
            tc.tile_pool(name="const", bufs=1) as cpool,
            tc.tile_pool(name="redu", bufs=12) as rpool,
            tc.tile_pool(name="astg", bufs=3) as apool,
            tc.tile_pool(name="wstg", bufs=2) as wspool,
            tc.tile_pool(name="xstg", bufs=2) as xspool,
            tc.tile_pool(name="w8", bufs=KB) as w8pool,
            tc.tile_pool(name="x8", bufs=KB * NSTRIPES) as x8pool,
            tc.tile_pool(name="psum", bufs=8, space="PSUM") as ppool,
            tc.tile_pool(name="ot", bufs=5) as opool,
            tc.tile_pool(name="dram", bufs=2, space="DRAM") as dpool,
        ):
            # ---- bias broadcast to all partitions -------------------------
            bias_row = cpool.tile([1, NSH], F16, tag="bias_row")
            nc.gpsimd.dma_start(bias_row[:], bias[:])
            bias_b = cpool.tile([128, NSH], F16, tag="bias_b")
            nc.gpsimd.partition_broadcast(bias_b[:], bias_row[:])

            # ---- distributed amax: abs-max of local x row-slice + w shard -
            # x and w chunks are interleaved so DMA + DVE pipeline from t=0.
            def amax_chunks(dram_ap, total_elems, tag):
                flat = dram_ap.rearrange("a k -> (a k)").rearrange(
                    "(p f) -> p f", p=128)
                per_part = total_elems // 128
                nchunk = max(1, per_part // 4096)
                csz = per_part // nchunk
                return flat, nchunk, csz

            def combine(partials, tag):
                while len(partials) > 1:
                    nxt = []
                    for i in range(0, len(partials) - 1, 2):
                        m = rpool.tile([128, 1], F32, tag=f"pm_{tag}",
                                       name=f"pmc_{tag}_{len(partials)}_{i}")
                        nc.vector.tensor_tensor(
                            m[:], partials[i][:], partials[i + 1][:],
                            op=mybir.AluOpType.max)
                        nxt.append(m)
                    if len(partials) % 2:
                        nxt.append(partials[-1])
                    partials = nxt
                return partials[0]

            xflat, xnc, xcsz = amax_chunks(xs, MS * K, "x")
            wflat, wnc, wcsz = amax_chunks(w, NSH * K, "w")
            xparts, wparts = [], []
            amax_dmas = []
            for c in range(max(xnc, wnc)):
                for (flat, n, csz, parts, tag) in (
                        (xflat, xnc, xcsz, xparts, "x"),
                        (wflat, wnc, wcsz, wparts, "w")):
                    if c >= n:
                        continue
                    stg = apool.tile([128, csz], F16, tag="astg",
                                     name=f"astg_{tag}_{c}")
                    amax_dmas.append(nc.gpsimd.dma_start(
                        stg[:], flat[:, c * csz:(c + 1) * csz]))
                    pm = rpool.tile([128, 1], F32, tag=f"pm_{tag}",
                                    name=f"pm_{tag}_{c}")
                    nc.vector.tensor_reduce(
                        pm[:], stg[:], axis=mybir.AxisListType.X,
                        op=mybir.AluOpType.max, apply_absolute_value=True)
                    parts.append(pm)
            px = combine(xparts, "x")
            pw = combine(wparts, "w")

            amax2 = rpool.tile([128, 2], F32, tag="amax2")
            nc.vector.tensor_copy(amax2[:, 0:1], px[:])
            nc.vector.tensor_copy(amax2[:, 1:2], pw[:])
            amax2r = rpool.tile([128, 2], F32, tag="amax2r")
            nc.gpsimd.partition_all_reduce(
                amax2r[:], amax2[:], channels=128,
                reduce_op=bass_isa.ReduceOp.max)

            # ---- global amax via AllReduce(max) over the 8 cores ----------
            bin_ = dpool.tile([1, 2], F32)
            bout = dpool.tile([1, 2], F32)
            nc.gpsimd.dma_start(bin_[:], amax2r[0:1, :])
            nc.gpsimd.collective_compute(
                "AllReduce", mybir.AluOpType.max,
                replica_groups=[list(range(NCORES))],
                ins=[bin_.opt()], outs=[bout.opt()])
            g = rpool.tile([1, 2], F32, tag="g")
            nc.gpsimd.dma_start(g[:], bout[:])
            nc.vector.tensor_scalar_max(g[:], g[:], EPS)
            gb = rpool.tile([128, 2], F32, tag="gb")
            nc.gpsimd.partition_broadcast(gb[:], g[:])

            # scales: s = 224 * (1/amax), dequant r = 1/s
            u2 = rpool.tile([128, 2], F32, tag="u2")
            nc.vector.reciprocal(u2[:], gb[:])
            s2 = rpool.tile([128, 2], F32, tag="s2")
            nc.vector.tensor_scalar_mul(s2[:], u2[:], DEV_FP8_MAX)
            inv2 = rpool.tile([128, 2], F32, tag="inv2")
            nc.vector.reciprocal(inv2[:], s2[:])
            sx, sw = s2[:, 0:1], s2[:, 1:2]
            rx, rw = inv2[:, 0:1], inv2[:, 1:2]

            # ---- weight: transpose-load, quantize to w8 k-block tiles -----
            from concourse.bass import _add_dep_helper
            last_amax = amax_dmas[-1]
            w8 = []
            for kb in range(KB):
                w8.append(w8pool.tile([128, 2 * NSH], F8, tag="w8",
                          name=f"w8_{kb}"))
            # batched transposes: one DMA covers WB k-chunks via a 3D dest
            # (extra dims extend the partition dim: dest[p, c, n] = k-row
            # 128c+p), amortizing the per-transfer HWDGE overhead.
            WB = 4
            for b in range(KCH // WB):
                wstg = wspool.tile([128, WB, NSH], F16, tag="wstg")
                nc.sync.dma_start(
                    wstg[:], w[:, b * WB * 128:(b + 1) * WB * 128],
                    transpose=True)
                for j in range(WB):
                    c = b * WB + j
                    dst = w8[c // 2][:, (c % 2) * NSH:(c % 2 + 1) * NSH]
                    nc.scalar.activation(dst, wstg[:, j, :],
                                         mybir.ActivationFunctionType.Copy,
                                         scale=sw)

            # ---- x: per-stripe transpose-load + quantize, then matmuls ----
            # All x8 tiles are allocated up front; stripe 0's fp16 staging
            # borrows the (still empty) x8 tiles of the last stripes as
            # scratch, giving the DMA engines a deep dependency-free runway
            # while the amax AllReduce is still in flight. WAR tracking
            # orders the later stripes' real writes after the scratch reads.
            # Non-uniform stripes: a wide first stripe (big transfers that
            # amortize overhead while the amax AllReduce is pending), then
            # halved later stripes so the final PE burst has less
            # un-overlappable tail. Transfer size stays ~1MB via XB=4096/sw.
            stripe_widths = [SW] * NSTRIPES
            stripe_m0 = [sum(stripe_widths[:i]) for i in range(len(stripe_widths))]
            x8_all = []
            for s, sw_s in enumerate(stripe_widths):
                x8_all.append([x8pool.tile([128, 2 * sw_s], F8, tag="x8",
                                           name=f"x8_{s}_{kb}")
                               for kb in range(KB)])
            for s, sw_s in enumerate(stripe_widths):
                x8 = x8_all[s]
                m0s = stripe_m0[s]
                XB = max(1, 4096 // sw_s)
                NB = KCH // XB
                for b in range(NB):
                    src = x[m0s:m0s + sw_s,
                            b * XB * 128:(b + 1) * XB * 128]
                    xstg = xspool.tile([128, XB, sw_s], F16, tag="xstg",
                                       name=f"xstg_{s}_{b}")[:]
                    nc.sync.dma_start(xstg, src, transpose=True)
                    for j in range(XB):
                        c = b * XB + j
                        dst = x8[c // 2][:, (c % 2) * sw_s:
                                         (c % 2 + 1) * sw_s]
                        if c % 2 == 0:
                            nc.vector.tensor_scalar(
                                dst, xstg[:, j, :], sx, None,
                                op0=mybir.AluOpType.mult)
                        else:
                            nc.scalar.activation(
                                dst, xstg[:, j, :],
                                mybir.ActivationFunctionType.Copy,
                                scale=sx)

                for mc in range(sw_s // 128):
                    ps = ppool.tile([128, NSH], F32, tag="ps")
                    if double_row:
                        for kb in range(KB):
                            lhsT = x8[kb].rearrange(
                                "p (i m) -> p i m", i=2)[
                                :, :, mc * 128:(mc + 1) * 128]
                            rhs = w8[kb].rearrange("p (i n) -> p i n", i=2)
                            nc.tensor.matmul(
                                ps[:], lhsT, rhs,
                                start=(kb == 0), stop=(kb == KB - 1),
                                perf_mode=mybir.MatmulPerfMode.DoubleRow)
                    else:
                        for kb in range(KB):
                            for i in range(2):
                                lhsT = x8[kb][:, i * sw_s + mc * 128:
                                              i * sw_s + (mc + 1) * 128]
                                rhs = w8[kb][:, i * NSH:(i + 1) * NSH]
                                nc.tensor.matmul(
                                    ps[:], lhsT, rhs,
                                    start=(kb == 0 and i == 0),
                                    stop=(kb == KB - 1 and i == 1))
                    ot = opool.tile([128, NSH], F16, tag="ot")
                    nc.vector.tensor_scalar(
                        ot[:], ps[:], rx, rw,
                        op0=mybir.AluOpType.mult, op1=mybir.AluOpType.mult)
                    nc.vector.tensor_tensor(ot[:], ot[:], bias_b[:],
                                            op=mybir.AluOpType.add)
                    m0 = m0s + mc * 128
                    nc.gpsimd.dma_start(out[m0:m0 + 128, :], ot[:])

    nc.compile()
    return nc


DOUBLE_ROW = True

_CACHE = {}


def _get_kernel(M, K, NSH, SW, double_row=None):
    if double_row is None:
        double_row = DOUBLE_ROW
    key = (M, K, NSH, SW, double_row)
    if key not in _CACHE:
        _CACHE[key] = build_kernel(M, K, NSH, SW, double_row)
    return _CACHE[key]


def kernel(x, weight, bias):
    M, K = x.shape
    N = weight.shape[0]
    NSH = N // NCORES
    SW = 2048 if M % 2048 == 0 else (1024 if M % 1024 == 0 else M // 4)
    nc = _get_kernel(M, K, NSH, SW)
    MS = M // NCORES

    x = np.asarray(x)
    weight = np.asarray(weight)
    bias = np.asarray(bias)
    in_maps = []
    for c in range(NCORES):
        in_maps.append({
            "x": x,
            "xs": np.ascontiguousarray(x[c * MS:(c + 1) * MS, :]),
            "w": np.ascontiguousarray(weight[c * NSH:(c + 1) * NSH, :]),
            "bias": np.ascontiguousarray(bias[c * NSH:(c + 1) * NSH]
                                         .reshape(1, NSH)),
        })
    # The axon terminal occasionally reports a stale NRT_EXEC_UNIT error from
    # a previous session on first use; a retry lands on a recovered device.
    last_err = None
    for _ in range(3):
        try:
            res = run_bass_kernel_spmd(nc, in_maps,
                                       core_ids=list(range(NCORES)))
            break
        except Exception as e:  # noqa: BLE001
            last_err = e
            time.sleep(2.0)
    else:
        raise last_err
    return np.concatenate([res.results[c]["out"] for c in range(NCORES)],
                          axis=1)



# revision 29
# speedup vs baseline: 1.8711x; 1.8711x over previous
"""FP8 dynamic-quantized linear (nn_FP8Linear) on 8 Trainium2 NeuronCores.

out = fp16((x_fp8 @ w_fp8.T) / (sx*sw)) + bias, with per-tensor dynamic
fp8-e4m3 quantization of x and weight (scale = FP8_MAX / amax).

Sharding: 2x4 tensor-parallel grid. x rows split in 2 halves (replicated
across the 4 cores of a row group); weight/bias split in 4 column slabs
(replicated across the 2 cores of a column group). Each core computes a
[M/2, N/4] output slab; the host stitches the 8 slabs (no output
collective needed). This halves per-core DMA vs out_features-only
sharding (24MB of fp16 loads instead of 36MB).

Global per-tensor amaxes (must match the reference exactly) come from a
"coverage" scheme: each core's FIRST-loaded 8MB -- a distinct quarter of
its x half (m-stripe 0 after a host-side m-roll) and a distinct n-half
of its w slab (after a host-side n-roll) -- is abs-max-reduced as it
lands in SBUF (split between the DVE and GpSimd engines so the
reduction keeps pace with the DMA). The 8 partial pairs are combined
with one tiny AllGather plus a local max; the union of the 8 coverage
sets is exactly x and w, so the result is the exact global amax and
quantization matches the reference bit-for-bit (modulo the power-of-2
scale trick below).

The Tile scheduler serializes DmaTranspose against collectives (they
share the DMA/XBAR path on hardware), so w is loaded in NATURAL layout
(plain DMA overlaps the collective) and transposed to k-major on the
otherwise-idle PE (matmul-transpose via an identity), with psum->SBUF
assembly copies on DVE/Act. x coverage is DMA-transposed before the
collective; the x remainder is DMA-transposed after the scale readback
(explicit dep) so it cannot delay the collective or the readback.

TRN fp8e4 (float8_e4m3) has max +-240 vs OCP e4m3fn's +-448, so the
device uses scale 224/amax == ref_scale/2: fp8 grids are self-similar
under powers of two, so device fp8 values are exactly half the
reference's, and the dequant multipliers absorb the factor of 4.
"""

import time

import numpy as np

import concourse.bacc as bacc
import concourse.bass as bass
import concourse.bass_isa as bass_isa
import concourse.mybir as mybir
import concourse.tile as tile
from concourse import masks
from concourse.bass import _add_dep_helper
from concourse.bass_utils import run_bass_kernel_spmd

F16 = mybir.dt.float16
F32 = mybir.dt.float32
F8 = mybir.dt.float8e4

NCORES = 8
RGRP, CGRP = 2, 4       # row groups (x halves) x col groups (w slabs)
EPS = 1e-12
# device-side quantization scale numerator: ref uses 448 (e4m3fn max); we use
# 224 so quantized values stay within TRN e4m3's +-240 normal range.
DEV_FP8_MAX = 224.0
DOUBLE_ROW = True
POOL_QUANT = False
WARMUP = 30


def build_kernel(M=4096, K=4096, NSH=1024, double_row=True,
                 pool_quant=POOL_QUANT, warmup=WARMUP, out_eng="pool",
                 deq="dve", cp_act=False, preload=True):
    """Build + compile the per-core bass program.

    Per-core shapes: x [M/2, K], w [NSH, K], out [M/2, NSH] with NSH=N/4.
    double_row: fp8 DoubleRow matmuls (2x PE throughput, ~1e-4 rel noise).
    warmup: number of discarded fp16 matmuls (gated on the last w load)
    bridging the PE p-state between the w transposes and the fp8 burst.
    pool_quant: also use the gpsimd (Pool) engine for fp16->fp8 quantize.
    """
    MH = M // RGRP            # 2048 token rows per core
    KB = K // 256             # 16 k-blocks (DoubleRow contracts 256/pass)
    NSTRIPES = 4
    SWM = MH // NSTRIPES      # 512-row m-stripes
    MCH = MH // 128           # 16 m-chunks per core
    KW = K // 4               # transfer k-width (1024)
    KCH = K // 128            # 32 k-chunks
    WNT = NSH // 128          # 8 natural w tiles
    assert MH % NSTRIPES == 0 and K % 256 == 0

    nc = bacc.Bacc("TRN2", target_bir_lowering=False, debug=False,
                   num_devices=NCORES)
    x = nc.dram_tensor("x", [MH, K], F16, kind="ExternalInput").ap()
    w = nc.dram_tensor("w", [NSH, K], F16, kind="ExternalInput").ap()
    bias = nc.dram_tensor("bias", [1, NSH], F16, kind="ExternalInput").ap()
    out = nc.dram_tensor("out", [MH, NSH], F16, kind="ExternalOutput").ap()

    # greedy engine balancers (ns/elem/partition + fixed overhead),
    # calibrated against observed TimelineSim slice durations
    cp_rate = {"v": 2.2 if cp_act else 0.72, "a": 1.0}  # psum->SBUF copies
    cp_load = {k: 0.0 for k in cp_rate}
    q_rate = {"v": 0.52, "a": 0.92}               # fp16->fp8 quantize
    if pool_quant:
        q_rate["p"] = 1.48
    q_fix = {"v": 60.0, "a": 150.0, "p": 150.0}
    # reserve DVE for dequant+bias, Act for out-DMA dispatch, Pool for smalls
    q_load = {"v": 0.0, "a": 0.0}
    if pool_quant:
        q_load["p"] = 0.0

    DVE_SHARE = 0.57          # coverage amax: DVE share vs gpsimd

    with tile.TileContext(nc) as tc:
        with (
            tc.tile_pool(name="const", bufs=1) as cpool,
            tc.tile_pool(name="redu", bufs=16) as rpool,
            tc.tile_pool(name="nat", bufs=4) as natpool,
            tc.tile_pool(name="wstg", bufs=4) as wspool,
            tc.tile_pool(name="xstg", bufs=6) as xspool,
            tc.tile_pool(name="w8", bufs=KB) as w8pool,
            tc.tile_pool(name="x8", bufs=KB + 2) as x8pool,
            tc.tile_pool(name="psum", bufs=3, space="PSUM") as ppool,
            tc.tile_pool(name="tp", bufs=2, space="PSUM") as tppool,
            tc.tile_pool(name="ot", bufs=4) as opool,
            tc.tile_pool(name="dram", bufs=2, space="DRAM") as dpool,
        ):
            # ---- constants ------------------------------------------------
            bias_row = cpool.tile([1, NSH], F16, tag="bias_row")
            nc.gpsimd.dma_start(bias_row[:], bias[:])
            bias_b = cpool.tile([128, NSH], F16, tag="bias_b")
            nc.gpsimd.partition_broadcast(bias_b[:], bias_row[:])
            ident = cpool.tile([128, 128], F16, tag="ident")
            masks.make_identity(nc, ident[:])

            # partial amaxes land in columns of shared accumulators; one
            # final reduce replaces a pairwise combine tree
            dax = rpool.tile([128, 8], F32, tag="dax")
            daw = rpool.tile([128, 8], F32, tag="daw")
            pax = rpool.tile([1, 8], F32, tag="pax")
            paw = rpool.tile([1, 8], F32, tag="paw")
            nc.gpsimd.memset(dax[:], 0.0)
            nc.gpsimd.memset(daw[:], 0.0)
            nc.gpsimd.memset(pax[:], 0.0)
            nc.gpsimd.memset(paw[:], 0.0)
            n_d = {"x": 0, "w": 0}

            def amax_of(flat_ap, free, tag):
                h = int(free * DVE_SHARE) & ~63
                da = dax if tag == "x" else daw
                pa = pax if tag == "x" else paw
                i = n_d[tag]
                n_d[tag] += 1
                nc.vector.tensor_reduce(
                    da[:, i:i + 1], flat_ap[:, 0:h],
                    axis=mybir.AxisListType.X,
                    op=mybir.AluOpType.max, apply_absolute_value=True)
                nc.gpsimd.tensor_reduce(
                    pa[:, i:i + 1], flat_ap[:, h:free],
                    axis=mybir.AxisListType.XYZWC,
                    op=mybir.AluOpType.max, apply_absolute_value=True)

            # ---- w natural loads + PE transposes into k-major wstg --------
            # Half-tiles [128 n, K/2] keep the load->transpose->reuse chain
            # fine-grained so DMA never waits on the PE. After the host
            # n-roll, tiles nt<4 are this core's distinct amax coverage.
            def cp(dst_ap, src_ap, elems):
                e = min(cp_load,
                        key=lambda k: cp_load[k] + elems * cp_rate[k])
                cp_load[e] += elems * cp_rate[e] + 250.0
                if e == "v":
                    nc.vector.tensor_copy(dst_ap, src_ap)
                else:
                    nc.scalar.activation(dst_ap, src_ap,
                                         mybir.ActivationFunctionType.Copy)

            wstg = [wspool.tile([128, 8, NSH], F16, tag="wstg",
                                name=f"wstg_{g}") for g in range(4)]
            wnat = {}

            def load_wnat(nt, h):
                nat = natpool.tile([128, K // 2], F16, tag="nat",
                                   name=f"wnat_{nt}_{h}")
                nc.sync.dma_start(
                    nat[:], w[nt * 128:(nt + 1) * 128,
                              h * (K // 2):(h + 1) * (K // 2)])
                wnat[(nt, h)] = nat
                if nt < 4:
                    amax_of(nat[:], K // 2, "w")
                for g in range(2):
                    pst = tppool.tile([128, 8, 128], F16, tag="tp",
                                      name=f"tp_{nt}_{h}_{g}")
                    for j in range(8):
                        c = 8 * g + j
                        nc.tensor.transpose(
                            pst[:, j, :], nat[:, c * 128:(c + 1) * 128],
                            ident[:])
                    cp(wstg[2 * h + g][:, 0:8, nt * 128:(nt + 1) * 128],
                       pst[:], 8 * 128)

            for nt in range(4):
                for h in range(2):
                    load_wnat(nt, h)

            # ---- x stripe-0 coverage: 4 transposed transfers [SWM, K/4] ---
            xstg = {}
            for t in range(4):
                stg = xspool.tile([128, KW // 128, SWM], F16, tag="xstg",
                                  name=f"xcov_{t}")
                nc.sync.dma_start(
                    stg[:], x[0:SWM, t * KW:(t + 1) * KW], transpose=True)
                xstg[(0, t)] = stg
                amax_of(stg[:].rearrange("p a b -> p (a b)"),
                        KW // 128 * SWM, "x")

            # ---- w rest (overlaps the collective: plain DMA) --------------
            for nt in range(4, WNT):
                for h in range(2):
                    load_wnat(nt, h)

            # ---- AllGather(concat) global amaxes --------------------------
            _hp = tc.high_priority()
            _hp.__enter__()
            amax2 = rpool.tile([128, 2], F32, tag="amax2")
            nc.vector.tensor_reduce(amax2[:, 0:1], dax[:],
                                    axis=mybir.AxisListType.X,
                                    op=mybir.AluOpType.max)
            nc.vector.tensor_reduce(amax2[:, 1:2], daw[:],
                                    axis=mybir.AxisListType.X,
                                    op=mybir.AluOpType.max)
            amax2r = rpool.tile([128, 2], F32, tag="amax2r")
            nc.gpsimd.partition_all_reduce(
                amax2r[:], amax2[:], channels=128,
                reduce_op=bass_isa.ReduceOp.max)
            p2 = rpool.tile([1, 2], F32, tag="p2")
            nc.vector.tensor_reduce(p2[:, 0:1], pax[:],
                                    axis=mybir.AxisListType.X,
                                    op=mybir.AluOpType.max)
            nc.vector.tensor_reduce(p2[:, 1:2], paw[:],
                                    axis=mybir.AxisListType.X,
                                    op=mybir.AluOpType.max)
            bin2 = rpool.tile([1, 2], F32, tag="bin2")
            nc.vector.tensor_tensor(bin2[:], amax2r[0:1, :], p2[:],
                                    op=mybir.AluOpType.max)

            bin_ = dpool.tile([1, 2], F32, name="bin_")
            bout = dpool.tile([1, 2 * NCORES], F32, name="bout")
            nc.gpsimd.dma_start(bin_[:], bin2[:])
            cc = nc.gpsimd.collective_compute(
                "AllGather", mybir.AluOpType.bypass,
                replica_groups=[list(range(NCORES))],
                ins=[bin_.opt()], outs=[bout.opt()])
            g16 = rpool.tile([1, 2 * NCORES], F32, tag="g16")
            g16_read = nc.gpsimd.dma_start(g16[:], bout[:])
            # gathered layout: [c0x, c0w, c1x, c1w, ...] -> max over cores
            gm = rpool.tile([1, 2], F32, tag="gm")
            nc.vector.tensor_reduce(
                gm[:], g16[:].rearrange("a (g t) -> a t g", t=2),
                axis=mybir.AxisListType.X, op=mybir.AluOpType.max)
            nc.vector.tensor_scalar_max(gm[:], gm[:], EPS)
            gb = rpool.tile([128, 2], F32, tag="gb")
            nc.gpsimd.partition_broadcast(gb[:], gm[:])

            # scales: s = 224/amax (quant), r = 1/s (dequant), r2 = rx*rw
            u2 = rpool.tile([128, 2], F32, tag="u2")
            nc.vector.reciprocal(u2[:], gb[:])
            s2 = rpool.tile([128, 2], F32, tag="s2")
            nc.vector.tensor_scalar_mul(s2[:], u2[:], DEV_FP8_MAX)
            inv2 = rpool.tile([128, 2], F32, tag="inv2")
            nc.vector.reciprocal(inv2[:], s2[:])
            r2 = rpool.tile([128, 1], F32, tag="r2")
            nc.vector.tensor_tensor(r2[:], inv2[:, 0:1], inv2[:, 1:2],
                                    op=mybir.AluOpType.mult)
            sx, sw = s2[:, 0:1], s2[:, 1:2]
            _hp.__exit__(None, None, None)

            # ---- x stripes 1-3: transposed loads AFTER the readback -------
            # (DmaTranspose serializes against the collective; gating these
            # on the readback keeps the collective + scales path clean.)
            for s in range(1, NSTRIPES):
                for q in range(4):
                    stg = xspool.tile([128, KW // 128, SWM], F16,
                                      tag="xstg", name=f"xstg_{s}_{q}")
                    d = nc.sync.dma_start(
                        stg[:], x[s * SWM:(s + 1) * SWM,
                                  q * KW:(q + 1) * KW],
                        transpose=True)
                    _add_dep_helper(d.ins, cc.ins, sync=True,
                                    reason="hold transposes off collective")
                    xstg[(s, q)] = stg

            # ---- PE p-state bridge: discarded fp16 matmuls ----------------
            if warmup:
                dps = ppool.tile([128, NSH], F32, tag="ps", name="dps")
                rhs = wnat[(WNT - 1, 1)][:, 0:512]
                lhsT = wnat[(WNT - 1, 1)][:, 512:640]
                for _ in range(warmup):
                    nc.tensor.matmul(dps[:, 0:512], lhsT, rhs,
                                     start=True, stop=True)

            # ---- quantize (greedy engine balance) -------------------------
            def quant(dst_ap, src_ap, scale_ap, elems):
                e = min(q_load,
                        key=lambda k: q_load[k] + elems * q_rate[k])
                q_load[e] += elems * q_rate[e] + q_fix[e]
                if e == "v":
                    nc.vector.tensor_scalar(dst_ap, src_ap, scale_ap, None,
                                            op0=mybir.AluOpType.mult)
                elif e == "a":
                    nc.scalar.activation(dst_ap, src_ap,
                                         mybir.ActivationFunctionType.Copy,
                                         scale=scale_ap)
                else:
                    nc.gpsimd.tensor_scalar(dst_ap, src_ap, scale_ap, None,
                                            op0=mybir.AluOpType.mult)

            w8 = [w8pool.tile([128, 2 * NSH], F8, tag="w8", name=f"w8_{kb}")
                  for kb in range(KB)]
            x8 = {}
            for s in range(NSTRIPES):
                for kb in range(KB):
                    x8[(s, kb)] = x8pool.tile([128, 2 * SWM], F8, tag="x8",
                                              name=f"x8_{s}_{kb}")

            def w_src(kb):
                t = kb // 4            # wstg tile (KW k each, 8 chunks)
                c = 2 * kb - 8 * t
                return wstg[t][:, c:c + 2, :].rearrange("p a b -> p (a b)")

            def x_src(s, kb):
                t = kb // 4            # xcov/xstg tile (KW k, 8 chunks)
                c = 2 * kb - 8 * t
                return xstg[(s, t)][:, c:c + 2, :].rearrange(
                    "p a b -> p (a b)")

            # first burst: interleave w8 and x8 stripe-0 in kb order so the
            # PE can accumulate (w8[kb], x8[0,kb]) pairs as they appear
            for kb in range(KB):
                quant(w8[kb][:], w_src(kb), sw, 2 * NSH)
                quant(x8[(0, kb)][:], x_src(0, kb), sx, 2 * SWM)
            if preload:
                # deq+bias land on DVE (and out dispatch on its engine)
                # during the stripe phase; bias the remaining quant splits
                q_load["v"] += 29500.0 if deq == "dve" else 19000.0
                if out_eng == "act":
                    q_load["a"] += 10000.0
                if pool_quant and out_eng == "pool":
                    q_load["p"] += 16000.0

            # ---- matmul sweep ---------------------------------------------
            for mc in range(MCH):
                s = mc // (MCH // NSTRIPES)
                lm = (mc % (MCH // NSTRIPES)) * 128
                if mc % (MCH // NSTRIPES) == 0 and s > 0:
                    for kb in range(KB):
                        quant(x8[(s, kb)][:], x_src(s, kb), sx, 2 * SWM)
                ps = ppool.tile([128, NSH], F32, tag="ps")
                if double_row:
                    for kb in range(KB):
                        lhsT = x8[(s, kb)].rearrange(
                            "p (i m) -> p i m", i=2)[:, :, lm:lm + 128]
                        rhs = w8[kb].rearrange("p (i n) -> p i n", i=2)
                        # accumulation group must stay within one 2KB PSUM
                        # bank (512 fp32): run the two n-halves separately
                        for nh in range(2):
                            nc.tensor.matmul(
                                ps[:, nh * (NSH // 2):(nh + 1) * (NSH // 2)],
                                lhsT, rhs[:, :, nh * (NSH // 2):
                                          (nh + 1) * (NSH // 2)],
                                start=(kb == 0), stop=(kb == KB - 1),
                                perf_mode=mybir.MatmulPerfMode.DoubleRow)
                else:
                    for kb in range(KB):
                        for i in range(2):
                            lhsT = x8[(s, kb)][:, i * SWM + lm:
                                               i * SWM + lm + 128]
                            rhs = w8[kb][:, i * NSH:(i + 1) * NSH]
                            nc.tensor.matmul(
                                ps[:], lhsT, rhs,
                                start=(kb == 0 and i == 0),
                                stop=(kb == KB - 1 and i == 1))
                ot = opool.tile([128, NSH], F16, tag="ot")
                if deq == "split":
                    nc.vector.tensor_scalar(ot[:, 0:NSH // 2],
                                            ps[:, 0:NSH // 2], r2[:], None,
                                            op0=mybir.AluOpType.mult)
                    nc.scalar.activation(ot[:, NSH // 2:], ps[:, NSH // 2:],
                                         mybir.ActivationFunctionType.Copy,
                                         scale=r2[:])
                elif deq == "dve" or (deq == "alt" and mc % 2 == 1):
                    nc.vector.tensor_scalar(ot[:], ps[:], r2[:], None,
                                            op0=mybir.AluOpType.mult)
                else:
                    nc.scalar.activation(ot[:], ps[:],
                                         mybir.ActivationFunctionType.Copy,
                                         scale=r2[:])
                nc.vector.tensor_tensor(ot[:], ot[:], bias_b[:],
                                        op=mybir.AluOpType.add)
                if out_eng == "act":
                    nc.scalar.dma_start(out[mc * 128:(mc + 1) * 128, :],
                                        ot[:])
                elif out_eng == "pool":
                    nc.gpsimd.dma_start(out[mc * 128:(mc + 1) * 128, :],
                                        ot[:])
                else:
                    nc.sync.dma_start(out[mc * 128:(mc + 1) * 128, :], ot[:])

    nc.compile()
    return nc


_CACHE = {}


def _get_kernel(M=4096, K=4096, NSH=None, SW=None, double_row=None):
    """NSH/SW args accepted for compatibility; config is fixed internally."""
    key = (M, K)
    if key not in _CACHE:
        _CACHE[key] = build_kernel(M, K, NSH=K // CGRP,
                                   double_row=DOUBLE_ROW)
    return _CACHE[key]


def kernel(x, weight, bias):
    M, K = x.shape
    N = weight.shape[0]
    nc = _get_kernel(M, K)
    MH, NSH = M // RGRP, N // CGRP
    SH = MH // CGRP           # x m-roll unit (x coverage distinctness)
    NR = NSH // RGRP          # w n-roll unit (w coverage distinctness)

    x = np.asarray(x)
    weight = np.asarray(weight)
    bias = np.asarray(bias)
    in_maps = []
    for core in range(NCORES):
        r, c = divmod(core, CGRP)
        xh = np.roll(x[r * MH:(r + 1) * MH], -SH * c, axis=0)
        wq = np.roll(weight[c * NSH:(c + 1) * NSH], -NR * r, axis=0)
        bq = np.roll(bias[c * NSH:(c + 1) * NSH], -NR * r)
        in_maps.append({
            "x": np.ascontiguousarray(xh),
            "w": np.ascontiguousarray(wq),
            "bias": np.ascontiguousarray(bq.reshape(1, NSH)),
        })
    # The axon terminal occasionally reports a stale NRT_EXEC_UNIT error from
    # a previous session on first use; a retry lands on a recovered device.
    last_err = None
    for _ in range(3):
        try:
            res = run_bass_kernel_spmd(nc, in_maps,
                                       core_ids=list(range(NCORES)))
            break
        except Exception as e:  # noqa: BLE001
            last_err = e
            time.sleep(2.0)
    else:
        raise last_err
    full = np.empty((M, N), dtype=np.float16)
    for core in range(NCORES):
        r, c = divmod(core, CGRP)
        o = np.asarray(res.results[core]["out"])
        o = np.roll(o, (SH * c, NR * r), axis=(0, 1))
        full[r * MH:(r + 1) * MH, c * NSH:(c + 1) * NSH] = o
    return full
